# revision 18
# baseline (speedup 1.0000x reference)
"""Trainium2 Bass kernel for nn_ConvUnit (cimu bit-sliced int8 conv2d).

Reference computation:
  xq = int8(trunc(clip(x, -128, 127)))                    # [32,128,56,56]
  for i in 0..7:
    bit_i = (xq >> i) & 1                                  # {0,1}
    c_i   = conv2d_valid(bit_i, W)                         # [32,128,54,54]
    q_i   = clip(round_half_even(c_i / 2), -128, 127) * 2
    y    += q_i * (2^i  if i < 7 else -128)
  y += bias

Strategy (8 NeuronCores, data-parallel over batch, 4 images/core):
  * ONE fp32r matmul pass per bit plane for planes 0-6.  HW-probed:
    fp32r x fp32r matmul = RNE of each operand to exactly 12 mantissa
    bits (e8m12), running at the same ~1 col/cycle as bf16 for
    N>=256.  At 2^-13 weight error the misround probability per
    element-plane is ~4.5e-5 (HW-measured ~530 per plane of 11.9M),
    so plane i contributes sqrt(p*N*4^(i+1)/||y||^2) of rel-err:
    negligible for i<=5, ~6e-3 for plane 6.
  * Plane 7 (misround cost 256 dominates the budget) is computed
    EXACTLY with two accumulating passes in one PSUM group:
    hi = rne12(w*k/2) (host-replicated lattice, passes through the
    PE's rounding unchanged) + lo = w*k/2 - hi shipped in bf16
    (12+8+ bits ~= full f32 precision).  Mixed fp32r/bf16 matmuls in
    one accumulation group are compiler- and HW-clean.
    Total: 9 pass-equivalents vs the bf16 hi/lo baseline's 18.
    Measured rel-err 7.52e-3 (gate 2e-2), deterministic across runs.
  * Conv as 9 shifted matmuls (taps) accumulating in PSUM over
    strided [9 rows x 54 cols] windows of the [56,56] plane: exactly
    6 tiles of 486 PSUM columns per image, no garbage columns, fully
    contiguous output DMA.
  * round_half_even via the magic-constant trick: since clip never
    fires (checked on host: max_co sum|W|/2 << 127.5),
        u_i = RNE(z + M_i) - M_i  ==  k_i * round_half_even(c_i/2)
    with M_i = 1.5*2^23*|k_i|.  ACT does t = z + M_i (exact f32 add),
    DVE scalar_tensor_tensor fuses (t - M_i) + y.
  * Bit planes in f32r {0,1} (exact): plane 7 is (x <= -1) on DVE;
    planes 0-6 via exact trunc ladder -> int32 xq -> shift&and (DVE)
    -> convert (ACT).
  * Schedule: weights stream on the sync/SP hwdge DMA queue in
    per-plane chunks (first plane split 3x); image-0 x rides the ACT
    queue in 4 column-chunks with the trunc ladder chunked behind it,
    so the PE starts ~12us in and never waits on bits again.  DMA
    trigger instructions cost ~0.7us on their issuing engine, so
    weight triggers live on the otherwise-idle sync engine.
    Tensor engine occupancy ~95%; ~229ns per 486-col matmul is the
    DVFS-limited column rate of this part.
"""
import sys

sys.path.insert(0, "/opt/trn_rl_repo")

import numpy as np

import concourse.bass as bass
import concourse.tile as tile
from concourse import bacc, mybir
from concourse import bass_utils

N_CORES = 8
B, C, H, W = 32, 128, 56, 56
HO, WO = 54, 54
BPC = B // N_CORES            # images per core
NPIX_IN = H * W               # 3136
NPIX = HO * WO                # 2916 output positions / image (exact)
TILE_N = 486                  # 9 output rows x 54 -> exactly 6 tiles
ROWS_PER_TILE = 9
NTILES = NPIX // TILE_N       # 6
# plane 7 first: its bit plane is just (x <= -1), no trunc ladder needed,
# so matmuls start early; the ladder hides behind plane-7 matmuls
PORDER = [7, 0, 1, 2, 3, 4, 5, 6]

MAGIC = 12582912.0            # 1.5 * 2^23: RNE(z + MAGIC) - MAGIC == rhe(z)
# per-plane scale k_i applied to q (folded into weights as k_i/2)
KSCALE = [float(2 << i) for i in range(7)] + [-256.0]

AluOp = mybir.AluOpType
ActFn = mybir.ActivationFunctionType
F32 = mybir.dt.float32
F32R = mybir.dt.float32r
I32 = mybir.dt.int32
BF = mybir.dt.bfloat16


# planes computed with 2 fp32r passes (exact rne12 hi + residual lo):
# the PE rounds each operand to 12 mantissa bits (RNE, probed on HW), so
# hi = rne12(w) passes through unchanged and lo = w - hi (exact in f32)
# restores full f32 weight precision across the two accumulated passes.
HILO_PLANES = (7,)
NBLK = 8 * 9                   # one fp32r hi block per plane/tap
NLO = len(HILO_PLANES) * 9     # bf16 lo blocks for the exact planes


def _rne12(a: np.ndarray) -> np.ndarray:
    man, ex = np.frexp(a.astype(np.float64))
    return np.ldexp(np.round(man * 4096.0) / 4096.0, ex).astype(np.float32)


def _prep_weights(weight: np.ndarray):
    """-> ([128ci, NBLK*128co] f32 hi blocks, [128ci, NLO*128co] bf16 lo)."""
    import ml_dtypes
    w2 = weight.astype(np.float32) * np.float32(0.5)
    blocks, lo_blocks = [], []
    for slot, p in enumerate(PORDER):
        s = w2 * np.float32(KSCALE[p])
        if p in HILO_PLANES:
            hi = _rne12(s)
            lo = (s - hi).astype(ml_dtypes.bfloat16)
        else:
            hi, lo = s, None
        for tap in range(9):
            blocks.append(hi[:, :, tap // 3, tap % 3].transpose(1, 0))
            if lo is not None:
                lo_blocks.append(lo[:, :, tap // 3, tap % 3].transpose(1, 0))
    hi_out = np.ascontiguousarray(
        np.stack(blocks, axis=1).reshape(C, NBLK * C))
    lo_out = np.ascontiguousarray(
        np.stack(lo_blocks, axis=1).reshape(C, NLO * C))
    return hi_out, lo_out


def _build(need_clip: bool):
    nc = bacc.Bacc("TRN2", target_bir_lowering=False, debug=False,
                   num_devices=N_CORES)
    xs = nc.dram_tensor("xs", [BPC, C, NPIX_IN], F32, kind="ExternalInput").ap()
    wt = nc.dram_tensor("wt", [C, NBLK * C], F32R, kind="ExternalInput").ap()
    wl = nc.dram_tensor("wl", [C, NLO * C], BF, kind="ExternalInput").ap()
    bs = nc.dram_tensor("bs", [C, 1], F32, kind="ExternalInput").ap()
    out = nc.dram_tensor("out", [BPC, C, HO, WO], F32, kind="ExternalOutput").ap()

    with tile.TileContext(nc) as tc:
        with (
            tc.tile_pool(name="wpool", bufs=1) as wpool,
            tc.tile_pool(name="cpool", bufs=1) as cpool,
            tc.tile_pool(name="xpool", bufs=2) as xpool,
            tc.tile_pool(name="tpool", bufs=1) as tpool,
            tc.tile_pool(name="xqpool", bufs=2) as xqpool,
            tc.tile_pool(name="b32pool", bufs=1) as b32pool,
            tc.tile_pool(name="bitpool", bufs=3) as bitpool,
            tc.tile_pool(name="bitbpool", bufs=1) as bitbpool,
            tc.tile_pool(name="ypool", bufs=2) as ypool,
            tc.tile_pool(name="upool", bufs=6) as upool,
            tc.tile_pool(name="psum", bufs=8, space="PSUM") as pspool,
        ):
            wsb = wpool.tile([C, NBLK * C], F32R)
            # per-plane chunks, issued in processing order so each plane's
            # weights land just ahead of its matmuls
            cstart = 0
            for i, p in enumerate(PORDER):
                ncols = 9 * C
                # sync/SP hwdge queue: the sync engine has no compute to
                # block, and x image 0 rides the ACT queue in parallel.
                # First plane's chunk split in 3 so its first matmuls are
                # not gated on the full 1.2MB transfer.
                if i == 0:
                    splits = [1 * C, 2 * C, 3 * C, 3 * C]
                else:
                    splits = [ncols]
                off = 0
                for w_ in splits:
                    nc.sync.dma_start(wsb[:, cstart + off:cstart + off + w_],
                                      wt[:, cstart + off:cstart + off + w_])
                    off += w_
                cstart += ncols
            wlo = wpool.tile([C, NLO * C], BF)
            nc.sync.dma_start(wlo[:], wl[:])
            bsb = cpool.tile([C, 1], F32)
            nc.sync.dma_start(bsb[:], bs[:])

            for img in range(BPC):
                xt = xpool.tile([C, NPIX_IN], F32, tag="x")
                bit7 = bitpool.tile([C, NPIX_IN], F32R, tag="bit")
                at = tpool.tile([C, NPIX_IN], F32, tag="ta")   # |c|
                st = tpool.tile([C, NPIX_IN], F32, tag="ts")   # sign(c)
                xq = xqpool.tile([C, NPIX_IN], I32, tag="xq")
                # image 0: 4-way column chunking pipelines DMA -> bit7 ->
                # trunc ladder so the first matmuls and plane-0 bits aren't
                # gated on the full 1.6MB x transfer + 20us ladder chain.
                # Later images are prefetched far ahead; one DMA suffices.
                if img == 0:
                    bounds = [0, 616, 1456, 2296, NPIX_IN]
                else:
                    bounds = [0, NPIX_IN]
                for cc in range(len(bounds) - 1):
                    sl = slice(bounds[cc], bounds[cc + 1])
                    if img == 0:
                        # ACT hwdge queue, in parallel with weights on sync
                        nc.scalar.dma_start(xt[:, sl], xs[img][:, sl])
                    else:
                        nc.sync.dma_start(xt[:, sl], xs[img][:, sl])
                    # plane 7 bits straight from x: b7 = (x <= -1)
                    nc.vector.tensor_scalar(bit7[:, sl], xt[:, sl], -1.0, None,
                                            AluOp.is_le)
                    # exact trunc-toward-zero ladder: xq = trunc(clip(x))
                    # c = min(max(x, -128), 127)   (in place in xt)
                    nc.vector.tensor_scalar(xt[:, sl], xt[:, sl], -128.0, 127.0,
                                            AluOp.max, AluOp.min)
                    nc.scalar.activation(at[:, sl], xt[:, sl], ActFn.Abs)
                    nc.scalar.activation(st[:, sl], xt[:, sl], ActFn.Sign)
                    # f = rhe(|c|)   (reuse xt)
                    nc.vector.tensor_scalar(xt[:, sl], at[:, sl], MAGIC, MAGIC,
                                            AluOp.add, AluOp.subtract)
                    # g = (f > |c|)  (into at; at dead after)
                    nc.vector.tensor_tensor(at[:, sl], xt[:, sl], at[:, sl],
                                            AluOp.is_gt)
                    # floor(|c|) = f - g   (into xt)
                    nc.vector.tensor_tensor(xt[:, sl], xt[:, sl], at[:, sl],
                                            AluOp.subtract)
                    # trunc(c) = floor(|c|) * sign(c)  (into xt)
                    nc.vector.tensor_tensor(xt[:, sl], xt[:, sl], st[:, sl],
                                            AluOp.mult)
                    # int32 convert (exact: integer-valued input)
                    nc.vector.tensor_copy(xq[:, sl], xt[:, sl])

                yt = ypool.tile([C, NPIX], F32, tag="y")
                bitw = None

                for slot, plane in enumerate(PORDER):
                    blk0 = slot * 9
                    lo0 = 9 * sum(1 for q in PORDER[:slot] if q in HILO_PLANES)
                    bitb = None
                    if plane == 7:
                        bit = bit7
                        bitb = bitbpool.tile([C, NPIX_IN], BF, tag="bitb")
                        nc.scalar.copy(bitb[:], bit7[:])
                    else:
                        # ---- bit plane: ((xq >> plane) & 1) as f32r ----
                        # first ladder-dependent slot of image 0 chunked so
                        # its tile-0 matmuls start as soon as the low rows
                        # of the bit plane are ready
                        b32 = b32pool.tile([C, NPIX_IN], I32, tag="b32")
                        bit = bitpool.tile([C, NPIX_IN], F32R, tag="bit")
                        nbch = 4 if (img == 0 and slot == 1) else 1
                        bsz = NPIX_IN // nbch
                        for bc in range(nbch):
                            bsl = slice(bc * bsz, (bc + 1) * bsz)
                            nc.vector.tensor_scalar(b32[:, bsl], xq[:, bsl],
                                                    plane, 1,
                                                    AluOp.logical_shift_right,
                                                    AluOp.bitwise_and)
                            nc.scalar.copy(bit[:, bsl], b32[:, bsl])
                        if plane in HILO_PLANES:
                            bitb = bitbpool.tile([C, NPIX_IN], BF, tag="bitb")
                            nc.scalar.copy(bitb[:], b32[:])

                    hilo = plane in HILO_PLANES
                    mag = MAGIC * abs(KSCALE[plane])
                    # [p, 56, 56] view for strided 9x54 tap windows
                    bitw = bit[:].rearrange("p (h w) -> p h w", w=W)
                    bitbw = (bitb[:].rearrange("p (h w) -> p h w", w=W)
                             if hilo else None)
                    for j in range(NTILES):
                        r0 = j * ROWS_PER_TILE
                        ps = pspool.tile([C, TILE_N], F32, tag="ps")
                        for tap in range(9):
                            kh, kw = tap // 3, tap % 3
                            mov = bitw[:, r0 + kh:r0 + kh + ROWS_PER_TILE,
                                       kw:kw + WO]
                            widx = blk0 + tap
                            nc.tensor.matmul(
                                ps[:],
                                wsb[:, widx * C:(widx + 1) * C],
                                mov,
                                start=(tap == 0),
                                stop=(tap == 8 and not hilo),
                            )
                        if hilo:
                            # bf16 lo-residual taps accumulate into the same
                            # PSUM group: rne12 hi + bf16 lo ~= exact f32 conv
                            for tap in range(9):
                                kh, kw = tap // 3, tap % 3
                                movb = bitbw[:, r0 + kh:r0 + kh + ROWS_PER_TILE,
                                             kw:kw + WO]
                                lidx = lo0 + tap
                                nc.tensor.matmul(
                                    ps[:],
                                    wlo[:, lidx * C:(lidx + 1) * C],
                                    movb,
                                    start=False,
                                    stop=(tap == 8),
                                )
                        yv = yt[:, j * TILE_N:(j + 1) * TILE_N]
                        if slot == 0:
                            # y = rhe(psum) * k  directly from PSUM on DVE
                            nc.vector.tensor_scalar(yv, ps[:], mag, mag,
                                                    AluOp.add, AluOp.subtract)
                        else:
                            # ACT: t = psum + M   (RNE -> rounds to mult of k)
                            ut = upool.tile([C, TILE_N], F32, tag="u")
                            nc.scalar.activation(ut[:], ps[:],
                                                 ActFn.Copy, bias=mag)
                            if need_clip:
                                lok, hik = ((-128.0, 127.0)
                                            if KSCALE[plane] > 0 else (-127.0, 128.0))
                                nc.vector.tensor_scalar(
                                    ut[:], ut[:],
                                    mag + lok * abs(KSCALE[plane]),
                                    mag + hik * abs(KSCALE[plane]),
                                    AluOp.max, AluOp.min)
                            # y = (t - M) + y   fused on DVE
                            nc.vector.scalar_tensor_tensor(
                                yv, ut[:], mag, yv,
                                AluOp.subtract, AluOp.add)
                        if slot == 7:
                            # last plane: bias + per-tile writeout (tiles are
                            # row-aligned: 9 output rows each, contiguous)
                            nc.vector.tensor_scalar(yv, yv, bsb[:, 0:1], None,
                                                    AluOp.add)
                            r0 = j * ROWS_PER_TILE
                            ysrc = yt[:].rearrange("p (h w) -> p h w", w=WO)[
                                :, r0:r0 + ROWS_PER_TILE, :]
                            nc.sync.dma_start(out[img][:, r0:r0 + ROWS_PER_TILE, :],
                                              ysrc)

    nc.compile()
    return nc


_CACHE = {}


def _get_nc(need_clip: bool):
    if need_clip not in _CACHE:
        _CACHE[need_clip] = _build(need_clip)
    return _CACHE[need_clip]


def kernel(x: np.ndarray, weight: np.ndarray, bias: np.ndarray,
           _trace: bool = False):
    x = np.ascontiguousarray(x, dtype=np.float32)
    weight = np.ascontiguousarray(weight, dtype=np.float32)
    bias = np.ascontiguousarray(bias, dtype=np.float32)

    w_host, w_lo_host = _prep_weights(weight)
    # clip in the reference only fires if |conv/2| can reach 127.5
    need_clip = float(np.abs(weight).sum(axis=(1, 2, 3)).max()) * 0.5 >= 127.4
    nc = _get_nc(need_clip)

    bs_host = bias.reshape(C, 1)
    xr = x.reshape(B, C, NPIX_IN)
    in_maps = []
    for c in range(N_CORES):
        in_maps.append({
            "xs": np.ascontiguousarray(xr[c * BPC:(c + 1) * BPC]),
            "wt": w_host,
            "wl": w_lo_host,
            "bs": bs_host,
        })

    res = bass_utils.run_bass_kernel_spmd(
        nc, in_maps, core_ids=list(range(N_CORES)), trace=_trace)

    y = np.concatenate([res.results[c]["out"] for c in range(N_CORES)], axis=0)
    if _trace:
        kernel._last_results = res
    return y


if __name__ == "__main__":
    np.random.seed(0)
    x = (np.random.randn(B, C, H, W) * 60).astype(np.float32)
    w = (np.random.randn(C, C, 3, 3) * 0.05).astype(np.float32)
    b = np.random.randn(C).astype(np.float32)
    y = kernel(x, w, b)
    print("out", y.shape, y.dtype)


# revision 19
# speedup vs baseline: 1.0016x; 1.0016x over previous
"""Trainium2 Bass kernel for nn_ConvUnit (cimu bit-sliced int8 conv2d).

Reference computation:
  xq = int8(trunc(clip(x, -128, 127)))                    # [32,128,56,56]
  for i in 0..7:
    bit_i = (xq >> i) & 1                                  # {0,1}
    c_i   = conv2d_valid(bit_i, W)                         # [32,128,54,54]
    q_i   = clip(round_half_even(c_i / 2), -128, 127) * 2
    y    += q_i * (2^i  if i < 7 else -128)
  y += bias

Strategy (8 NeuronCores, data-parallel over batch, 4 images/core):
  * ONE fp32r matmul pass per bit plane for planes 0-6.  HW-probed:
    fp32r x fp32r matmul = RNE of each operand to exactly 12 mantissa
    bits (e8m12), running at the same ~1 col/cycle as bf16 for
    N>=256.  At 2^-13 weight error the misround probability per
    element-plane is ~4.5e-5 (HW-measured ~530 per plane of 11.9M),
    so plane i contributes sqrt(p*N*4^(i+1)/||y||^2) of rel-err:
    negligible for i<=5, ~6e-3 for plane 6.
  * Plane 7 (misround cost 256 dominates the budget) is computed
    EXACTLY with two accumulating passes in one PSUM group:
    hi = rne12(w*k/2) (host-replicated lattice, passes through the
    PE's rounding unchanged) + lo = w*k/2 - hi shipped in bf16
    (12+8+ bits ~= full f32 precision).  Mixed fp32r/bf16 matmuls in
    one accumulation group are compiler- and HW-clean.
    Total: 9 pass-equivalents vs the bf16 hi/lo baseline's 18.
    Measured rel-err 7.52e-3 (gate 2e-2), deterministic across runs.
  * Conv as 9 shifted matmuls (taps) accumulating in PSUM over
    strided [9 rows x 54 cols] windows of the [56,56] plane: exactly
    6 tiles of 486 PSUM columns per image, no garbage columns, fully
    contiguous output DMA.
  * round_half_even via the magic-constant trick: since clip never
    fires (checked on host: max_co sum|W|/2 << 127.5),
        u_i = RNE(z + M_i) - M_i  ==  k_i * round_half_even(c_i/2)
    with M_i = 1.5*2^23*|k_i|.  ACT does t = z + M_i (exact f32 add),
    DVE scalar_tensor_tensor fuses (t - M_i) + y.
  * Bit planes in f32r {0,1} (exact): plane 7 is (x <= -1) on DVE;
    planes 0-6 via exact trunc ladder -> int32 xq -> shift&and (DVE)
    -> convert (ACT).
  * Schedule: weights stream on the sync/SP hwdge DMA queue in
    per-plane chunks (first plane split 3x); image-0 x rides the ACT
    queue in 4 column-chunks with the trunc ladder chunked behind it,
    so the PE starts ~12us in and never waits on bits again.  DMA
    trigger instructions cost ~0.7us on their issuing engine, so
    weight triggers live on the otherwise-idle sync engine.
    Tensor engine occupancy ~95%; ~229ns per 486-col matmul is the
    DVFS-limited column rate of this part.
"""
import sys

sys.path.insert(0, "/opt/trn_rl_repo")

import numpy as np

import concourse.bass as bass
import concourse.tile as tile
from concourse import bacc, mybir
from concourse import bass_utils

N_CORES = 8
B, C, H, W = 32, 128, 56, 56
HO, WO = 54, 54
BPC = B // N_CORES            # images per core
NPIX_IN = H * W               # 3136
NPIX = HO * WO                # 2916 output positions / image (exact)
TILE_N = 486                  # 9 output rows x 54 -> exactly 6 tiles
ROWS_PER_TILE = 9
NTILES = NPIX // TILE_N       # 6
# plane 7 first: its bit plane is just (x <= -1), no trunc ladder needed,
# so matmuls start early; the ladder hides behind plane-7 matmuls
PORDER = [7, 0, 1, 2, 3, 4, 5, 6]

MAGIC = 12582912.0            # 1.5 * 2^23: RNE(z + MAGIC) - MAGIC == rhe(z)
# per-plane scale k_i applied to q (folded into weights as k_i/2)
KSCALE = [float(2 << i) for i in range(7)] + [-256.0]

AluOp = mybir.AluOpType
ActFn = mybir.ActivationFunctionType
F32 = mybir.dt.float32
F32R = mybir.dt.float32r
I32 = mybir.dt.int32
BF = mybir.dt.bfloat16


# planes computed with 2 fp32r passes (exact rne12 hi + residual lo):
# the PE rounds each operand to 12 mantissa bits (RNE, probed on HW), so
# hi = rne12(w) passes through unchanged and lo = w - hi (exact in f32)
# restores full f32 weight precision across the two accumulated passes.
HILO_PLANES = (7,)
NBLK = 8 * 9                   # one fp32r hi block per plane/tap
NLO = len(HILO_PLANES) * 9     # bf16 lo blocks for the exact planes


def _rne12(a: np.ndarray) -> np.ndarray:
    man, ex = np.frexp(a.astype(np.float64))
    return np.ldexp(np.round(man * 4096.0) / 4096.0, ex).astype(np.float32)


def _prep_weights(weight: np.ndarray):
    """-> ([128ci, NBLK*128co] f32 hi blocks, [128ci, NLO*128co] bf16 lo)."""
    import ml_dtypes
    w2 = weight.astype(np.float32) * np.float32(0.5)
    blocks, lo_blocks = [], []
    for slot, p in enumerate(PORDER):
        s = w2 * np.float32(KSCALE[p])
        if p in HILO_PLANES:
            hi = _rne12(s)
            lo = (s - hi).astype(ml_dtypes.bfloat16)
        else:
            hi, lo = s, None
        for tap in range(9):
            blocks.append(hi[:, :, tap // 3, tap % 3].transpose(1, 0))
            if lo is not None:
                lo_blocks.append(lo[:, :, tap // 3, tap % 3].transpose(1, 0))
    hi_out = np.ascontiguousarray(
        np.stack(blocks, axis=1).reshape(C, NBLK * C))
    lo_out = np.ascontiguousarray(
        np.stack(lo_blocks, axis=1).reshape(C, NLO * C))
    return hi_out, lo_out


def _build(need_clip: bool):
    nc = bacc.Bacc("TRN2", target_bir_lowering=False, debug=False,
                   num_devices=N_CORES)
    xs = nc.dram_tensor("xs", [BPC, C, NPIX_IN], F32, kind="ExternalInput").ap()
    wt = nc.dram_tensor("wt", [C, NBLK * C], F32R, kind="ExternalInput").ap()
    wl = nc.dram_tensor("wl", [C, NLO * C], BF, kind="ExternalInput").ap()
    bs = nc.dram_tensor("bs", [C, 1], F32, kind="ExternalInput").ap()
    out = nc.dram_tensor("out", [BPC, C, HO, WO], F32, kind="ExternalOutput").ap()

    with tile.TileContext(nc) as tc:
        with (
            tc.tile_pool(name="wpool", bufs=1) as wpool,
            tc.tile_pool(name="cpool", bufs=1) as cpool,
            tc.tile_pool(name="xpool", bufs=2) as xpool,
            tc.tile_pool(name="tpool", bufs=1) as tpool,
            tc.tile_pool(name="xqpool", bufs=2) as xqpool,
            tc.tile_pool(name="b32pool", bufs=1) as b32pool,
            tc.tile_pool(name="bitpool", bufs=3) as bitpool,
            tc.tile_pool(name="bitbpool", bufs=1) as bitbpool,
            tc.tile_pool(name="ypool", bufs=2) as ypool,
            tc.tile_pool(name="upool", bufs=6) as upool,
            tc.tile_pool(name="psum", bufs=8, space="PSUM") as pspool,
        ):
            wsb = wpool.tile([C, NBLK * C], F32R)
            # per-plane chunks, issued in processing order so each plane's
            # weights land just ahead of its matmuls
            cstart = 0
            for i, p in enumerate(PORDER):
                ncols = 9 * C
                # sync/SP hwdge queue: the sync engine has no compute to
                # block, and x image 0 rides the ACT queue in parallel.
                # First plane's chunk split in 3 so its first matmuls are
                # not gated on the full 1.2MB transfer.
                nsub = 3 if i == 0 else 1
                sub = ncols // nsub
                for k in range(nsub):
                    nc.sync.dma_start(
                        wsb[:, cstart + k * sub:cstart + (k + 1) * sub],
                        wt[:, cstart + k * sub:cstart + (k + 1) * sub])
                cstart += ncols
            wlo = wpool.tile([C, NLO * C], BF)
            nc.sync.dma_start(wlo[:], wl[:])
            bsb = cpool.tile([C, 1], F32)
            nc.sync.dma_start(bsb[:], bs[:])

            for img in range(BPC):
                xt = xpool.tile([C, NPIX_IN], F32, tag="x")
                bit7 = bitpool.tile([C, NPIX_IN], F32R, tag="bit")
                at = tpool.tile([C, NPIX_IN], F32, tag="ta")   # |c|
                st = tpool.tile([C, NPIX_IN], F32, tag="ts")   # sign(c)
                xq = xqpool.tile([C, NPIX_IN], I32, tag="xq")
                # image 0: 4-way column chunking pipelines DMA -> bit7 ->
                # trunc ladder so the first matmuls and plane-0 bits aren't
                # gated on the full 1.6MB x transfer + 20us ladder chain.
                # Later images are prefetched far ahead; one DMA suffices.
                NCH = 4 if img == 0 else 1
                csz = NPIX_IN // NCH
                for cc in range(NCH):
                    sl = slice(cc * csz, (cc + 1) * csz)
                    if img == 0:
                        # ACT hwdge queue, in parallel with weights on sync
                        nc.scalar.dma_start(xt[:, sl], xs[img][:, sl])
                    else:
                        nc.sync.dma_start(xt[:, sl], xs[img][:, sl])
                    # plane 7 bits straight from x: b7 = (x <= -1)
                    nc.vector.tensor_scalar(bit7[:, sl], xt[:, sl], -1.0, None,
                                            AluOp.is_le)
                    # exact trunc-toward-zero ladder: xq = trunc(clip(x))
                    # c = min(max(x, -128), 127)   (in place in xt)
                    nc.vector.tensor_scalar(xt[:, sl], xt[:, sl], -128.0, 127.0,
                                            AluOp.max, AluOp.min)
                    nc.scalar.activation(at[:, sl], xt[:, sl], ActFn.Abs)
                    nc.scalar.activation(st[:, sl], xt[:, sl], ActFn.Sign)
                    # f = rhe(|c|)   (reuse xt)
                    nc.vector.tensor_scalar(xt[:, sl], at[:, sl], MAGIC, MAGIC,
                                            AluOp.add, AluOp.subtract)
                    # g = (f > |c|)  (into at; at dead after)
                    nc.vector.tensor_tensor(at[:, sl], xt[:, sl], at[:, sl],
                                            AluOp.is_gt)
                    # floor(|c|) = f - g   (into xt)
                    nc.vector.tensor_tensor(xt[:, sl], xt[:, sl], at[:, sl],
                                            AluOp.subtract)
                    # trunc(c) = floor(|c|) * sign(c)  (into xt)
                    nc.vector.tensor_tensor(xt[:, sl], xt[:, sl], st[:, sl],
                                            AluOp.mult)
                    # int32 convert (exact: integer-valued input)
                    nc.vector.tensor_copy(xq[:, sl], xt[:, sl])

                yt = ypool.tile([C, NPIX], F32, tag="y")
                bitw = None

                for slot, plane in enumerate(PORDER):
                    blk0 = slot * 9
                    lo0 = 9 * sum(1 for q in PORDER[:slot] if q in HILO_PLANES)
                    bitb = None
                    if plane == 7:
                        bit = bit7
                        bitb = bitbpool.tile([C, NPIX_IN], BF, tag="bitb")
                        nc.scalar.copy(bitb[:], bit7[:])
                    else:
                        # ---- bit plane: ((xq >> plane) & 1) as f32r ----
                        # first ladder-dependent slot of image 0 chunked so
                        # its tile-0 matmuls start as soon as the low rows
                        # of the bit plane are ready
                        b32 = b32pool.tile([C, NPIX_IN], I32, tag="b32")
                        bit = bitpool.tile([C, NPIX_IN], F32R, tag="bit")
                        nbch = 4 if (img == 0 and slot == 1) else 1
                        bsz = NPIX_IN // nbch
                        for bc in range(nbch):
                            bsl = slice(bc * bsz, (bc + 1) * bsz)
                            nc.vector.tensor_scalar(b32[:, bsl], xq[:, bsl],
                                                    plane, 1,
                                                    AluOp.logical_shift_right,
                                                    AluOp.bitwise_and)
                            nc.scalar.copy(bit[:, bsl], b32[:, bsl])
                        if plane in HILO_PLANES:
                            bitb = bitbpool.tile([C, NPIX_IN], BF, tag="bitb")
                            nc.scalar.copy(bitb[:], b32[:])

                    hilo = plane in HILO_PLANES
                    mag = MAGIC * abs(KSCALE[plane])
                    # [p, 56, 56] view for strided 9x54 tap windows
                    bitw = bit[:].rearrange("p (h w) -> p h w", w=W)
                    bitbw = (bitb[:].rearrange("p (h w) -> p h w", w=W)
                             if hilo else None)
                    for j in range(NTILES):
                        r0 = j * ROWS_PER_TILE
                        ps = pspool.tile([C, TILE_N], F32, tag="ps")
                        for tap in range(9):
                            kh, kw = tap // 3, tap % 3
                            mov = bitw[:, r0 + kh:r0 + kh + ROWS_PER_TILE,
                                       kw:kw + WO]
                            widx = blk0 + tap
                            nc.tensor.matmul(
                                ps[:],
                                wsb[:, widx * C:(widx + 1) * C],
                                mov,
                                start=(tap == 0),
                                stop=(tap == 8 and not hilo),
                            )
                        if hilo:
                            # bf16 lo-residual taps accumulate into the same
                            # PSUM group: rne12 hi + bf16 lo ~= exact f32 conv
                            for tap in range(9):
                                kh, kw = tap // 3, tap % 3
                                movb = bitbw[:, r0 + kh:r0 + kh + ROWS_PER_TILE,
                                             kw:kw + WO]
                                lidx = lo0 + tap
                                nc.tensor.matmul(
                                    ps[:],
                                    wlo[:, lidx * C:(lidx + 1) * C],
                                    movb,
                                    start=False,
                                    stop=(tap == 8),
                                )
                        yv = yt[:, j * TILE_N:(j + 1) * TILE_N]
                        if slot == 0:
                            # y = rhe(psum) * k  directly from PSUM on DVE
                            nc.vector.tensor_scalar(yv, ps[:], mag, mag,
                                                    AluOp.add, AluOp.subtract)
                        else:
                            # ACT: t = psum + M   (RNE -> rounds to mult of k)
                            ut = upool.tile([C, TILE_N], F32, tag="u")
                            nc.scalar.activation(ut[:], ps[:],
                                                 ActFn.Copy, bias=mag)
                            if need_clip:
                                lok, hik = ((-128.0, 127.0)
                                            if KSCALE[plane] > 0 else (-127.0, 128.0))
                                nc.vector.tensor_scalar(
                                    ut[:], ut[:],
                                    mag + lok * abs(KSCALE[plane]),
                                    mag + hik * abs(KSCALE[plane]),
                                    AluOp.max, AluOp.min)
                            # y = (t - M) + y   fused on DVE
                            nc.vector.scalar_tensor_tensor(
                                yv, ut[:], mag, yv,
                                AluOp.subtract, AluOp.add)
                        if slot == 7:
                            # last plane: bias + per-tile writeout (tiles are
                            # row-aligned: 9 output rows each, contiguous)
                            nc.vector.tensor_scalar(yv, yv, bsb[:, 0:1], None,
                                                    AluOp.add)
                            r0 = j * ROWS_PER_TILE
                            ysrc = yt[:].rearrange("p (h w) -> p h w", w=WO)[
                                :, r0:r0 + ROWS_PER_TILE, :]
                            nc.sync.dma_start(out[img][:, r0:r0 + ROWS_PER_TILE, :],
                                              ysrc)

    nc.compile()
    return nc


_CACHE = {}


def _get_nc(need_clip: bool):
    if need_clip not in _CACHE:
        _CACHE[need_clip] = _build(need_clip)
    return _CACHE[need_clip]


def kernel(x: np.ndarray, weight: np.ndarray, bias: np.ndarray,
           _trace: bool = False):
    x = np.ascontiguousarray(x, dtype=np.float32)
    weight = np.ascontiguousarray(weight, dtype=np.float32)
    bias = np.ascontiguousarray(bias, dtype=np.float32)

    w_host, w_lo_host = _prep_weights(weight)
    # clip in the reference only fires if |conv/2| can reach 127.5
    need_clip = float(np.abs(weight).sum(axis=(1, 2, 3)).max()) * 0.5 >= 127.4
    nc = _get_nc(need_clip)

    bs_host = bias.reshape(C, 1)
    xr = x.reshape(B, C, NPIX_IN)
    in_maps = []
    for c in range(N_CORES):
        in_maps.append({
            "xs": np.ascontiguousarray(xr[c * BPC:(c + 1) * BPC]),
            "wt": w_host,
            "wl": w_lo_host,
            "bs": bs_host,
        })

    res = bass_utils.run_bass_kernel_spmd(
        nc, in_maps, core_ids=list(range(N_CORES)), trace=_trace)

    y = np.concatenate([res.results[c]["out"] for c in range(N_CORES)], axis=0)
    if _trace:
        kernel._last_results = res
    return y


if __name__ == "__main__":
    np.random.seed(0)
    x = (np.random.randn(B, C, H, W) * 60).astype(np.float32)
    w = (np.random.randn(C, C, 3, 3) * 0.05).astype(np.float32)
    b = np.random.randn(C).astype(np.float32)
    y = kernel(x, w, b)
    print("out", y.shape, y.dtype)
